# revision 6
# baseline (speedup 1.0000x reference)
"""Causal self-attention Bass kernel for 8 TRN2 NeuronCores.

Problem: B=4, T=2048, C=1024, H=16 heads, head_dim=64, fp32.
    q = x @ Wq.T ; k = x @ Wk.T ; v = x @ Wv.T          (per head)
    att = softmax(mask(q k^T / 8))
    y = att @ v ; out = y @ Wp.T

Sharding (8 cores): 4-way data parallel over batch x 2-way tensor
parallel over heads. Core c handles batch c//2 and heads 8*(c%2)..+8.
Wq/Wk/Wv column-parallel, Wp row-parallel; the partial outputs of the
two head-halves of each batch are summed on the host (the "all-reduce"
of row-parallel Wp).

Device dataflow (all transposed, so no on-chip transposes are needed):
    xT [C, T] (host-pretransposed) ->
    qT/kT = WqT.T-slices @ xT   [512, T]  (pairs of heads on partitions)
    v     = xT.T-tiles @ WvT    [T, 512]
    scoresT[k, q] = kT.T @ qT   (k on partitions -> softmax sum over k
                                 via a ones-column appended to v)
    expT = exp(0.125 * scoresT) (no max subtraction: scores ~ N(0, 0.4))
    yT[d, q] (+ row of sums) = v_aug.T @ expT, accumulated over k tiles
    out[t, c] = yT.T-tiles @ WpT, accumulated over local j

Everything computes in float32r (TensorE full rate at N>=256,
~1.5e-4 rel err vs fp32).
"""

from contextlib import ExitStack

import numpy as np

import concourse.bass as bass
import concourse.tile as tile
from concourse import bacc, mybir

F32 = mybir.dt.float32
F32R = mybir.dt.float32r

B, T, C, H, D = 4, 2048, 1024, 16, 64
NCORES = 8
JL = 512            # local j dims per core (8 heads * 64)
NPAIR = 4           # local head pairs
CI = C // 128       # 8 c-tiles
NT = T // 128       # 16 t/k tiles
NQC = T // 512      # 4 q chunks
NEG = -1.0e30

_CACHED_NC = None


def build_nc():
    nc = bacc.Bacc(None)

    xT = nc.dram_tensor("xT", [C, T], F32R, kind="ExternalInput")
    wqT = nc.dram_tensor("wqT", [C, JL], F32R, kind="ExternalInput")
    wkT = nc.dram_tensor("wkT", [C, JL], F32R, kind="ExternalInput")
    wvT = nc.dram_tensor("wvT", [C, JL], F32R, kind="ExternalInput")
    wpT = nc.dram_tensor("wpT", [JL, C], F32R, kind="ExternalInput")
    out = nc.dram_tensor("out", [T, C], F32, kind="ExternalOutput")
    # bounce buffer for broadcasting softmax reciprocals across partitions
    rcd = nc.dram_tensor("rcd", [NPAIR, NQC, 2, 512], F32)

    xT_r = xT.rearrange("(ci p) t -> p ci t", p=128)
    wq_r = wqT.rearrange("(ci p) j -> p ci j", p=128)
    wk_r = wkT.rearrange("(ci p) j -> p ci j", p=128)
    wv_r = wvT.rearrange("(ci p) j -> p ci j", p=128)
    wp_r = wpT.rearrange("(ji p) c -> p ji c", p=128)

    with tile.TileContext(nc) as tc, ExitStack() as ctx:
        pm = ctx.enter_context(tc.tile_pool(name="pm", bufs=1))
        qkp = ctx.enter_context(tc.tile_pool(name="qkp", bufs=1))
        expp = ctx.enter_context(tc.tile_pool(name="expp", bufs=3))
        bcp = ctx.enter_context(tc.tile_pool(name="bcp", bufs=1))
        rcp = ctx.enter_context(tc.tile_pool(name="rcp", bufs=1))
        stp = ctx.enter_context(tc.tile_pool(name="stp", bufs=2))
        gp = ctx.enter_context(tc.tile_pool(name="gp", bufs=2, space="PSUM"))
        yp = ctx.enter_context(tc.tile_pool(name="yp", bufs=4, space="PSUM"))

        # v with a ones column appended per head: [k-tile, head, 65]
        v_sb = pm.tile([128, NT, 8, D + 1], F32R)
        # causal triangle mask for diagonal 128x128 blocks of scoresT:
        # mask[p, q] = 0 where q >= p else -1e30  (k on partitions)
        mask = pm.tile([128, 128], F32)
        nc.gpsimd.memset(mask[:], 0.0)
        nc.gpsimd.affine_select(
            out=mask[:],
            in_=mask[:],
            compare_op=mybir.AluOpType.is_ge,
            fill=NEG,
            base=0,
            pattern=[[1, 128]],
            channel_multiplier=-1,
        )
        ones_col = pm.tile([128, NT, 8, 1], F32)
        nc.vector.memset(ones_col[:], 1.0)
        nc.vector.tensor_copy(v_sb[:, :, :, D : D + 1], ones_col[:])

        qT_all = qkp.tile([128, NPAIR, T], F32R, tag="qT_all")
        kT_all = qkp.tile([128, NPAIR, T], F32R, tag="kT_all")

        # ---- phase 1: projections (stream xT by t-chunks) -----------------
        with (
            tc.tile_pool(name="ph1w", bufs=1) as wpool,
            tc.tile_pool(name="ph1x", bufs=2) as xpool,
        ):
            wq_sb = wpool.tile([128, CI, JL], F32R, tag="wq")
            wk_sb = wpool.tile([128, CI, JL], F32R, tag="wk")
            wv_sb = wpool.tile([128, CI, JL], F32R, tag="wv")
            nc.sync.dma_start(wq_sb[:], wq_r[:])
            nc.sync.dma_start(wk_sb[:], wk_r[:])
            nc.sync.dma_start(wv_sb[:], wv_r[:])

            for tch in range(NQC):
                ts_ = slice(tch * 512, tch * 512 + 512)
                xt = xpool.tile([128, CI, 512], F32R, tag="xt")
                nc.sync.dma_start(xt[:], xT_r[:, :, ts_])

                for w_sb, dst in ((wq_sb, qT_all), (wk_sb, kT_all)):
                    for pr in range(NPAIR):
                        acc = gp.tile([128, 2, 512], F32, tag="g")
                        for ci in range(CI):
                            nc.tensor.matmul(
                                acc[:, 0, :],
                                w_sb[:, ci, pr * 128 : pr * 128 + 128],
                                xt[:, ci, :],
                                start=(ci == 0),
                                stop=(ci == CI - 1),
                            )
                        nc.vector.tensor_copy(dst[:, pr, ts_], acc[:, 0, :])

                for tl in range(4):
                    ti = tch * 4 + tl
                    acc = gp.tile([128, 2, 512], F32, tag="g")
                    for ci in range(CI):
                        nc.tensor.matmul(
                            acc[:, 0, :],
                            xt[:, ci, tl * 128 : tl * 128 + 128],
                            wv_sb[:, ci, :],
                            start=(ci == 0),
                            stop=(ci == CI - 1),
                        )
                    nc.vector.tensor_copy(
                        v_sb[:, ti, :, 0:D],
                        acc[:, 0, :].rearrange("p (h d) -> p h d", d=D),
                    )

        # ---- phase 2: attention + output projection ----------------------
        with (
            tc.tile_pool(name="ph2", bufs=1) as p2,
            tc.tile_pool(name="outp", bufs=3) as outp,
        ):
            wp_sb = p2.tile([128, NPAIR, C], F32R, tag="wp")
            nc.sync.dma_start(wp_sb[:], wp_r[:])
            yT_all = p2.tile([128, NPAIR, T], F32R, tag="yT")

            for pr in range(NPAIR):
                qlo = qT_all[0:64, pr, :]
                qhi = qT_all[64:128, pr, :]
                klo = kT_all[0:64, pr, :]
                khi = kT_all[64:128, pr, :]
                for qc in range(NQC):
                    nkt = 4 * qc + 4
                    qs = slice(qc * 512, qc * 512 + 512)
                    yA = yp.tile([D + 1, 512], F32, tag="y")
                    yB = yp.tile([D + 1, 512], F32, tag="y")
                    for kt in range(nkt):
                        dt = kt - 4 * qc
                        ks = slice(kt * 128, kt * 128 + 128)
                        g = gp.tile([128, 2, 512], F32, tag="g")
                        nc.tensor.matmul(
                            g[:, 0, :], klo[:, ks], qlo[:, qs], start=True, stop=True
                        )
                        nc.tensor.matmul(
                            g[:, 1, :], khi[:, ks], qhi[:, qs], start=True, stop=True
                        )
                        if dt >= 0:
                            bs = slice(dt * 128, dt * 128 + 128)
                            nc.vector.tensor_add(g[:, 0, bs], g[:, 0, bs], mask[:])
                            nc.vector.tensor_add(g[:, 1, bs], g[:, 1, bs], mask[:])
                        e = expp.tile([128, 2, 512], F32R, tag="e")
                        nc.scalar.activation(
                            e[:], g[:], mybir.ActivationFunctionType.Exp, scale=0.125
                        )
                        lo = dt * 128 if dt > 0 else 0
                        nc.tensor.matmul(
                            yA[:, lo:512],
                            v_sb[:, kt, 2 * pr, :],
                            e[:, 0, lo:512],
                            start=(kt == 0),
                            stop=(kt == nkt - 1),
                        )
                        nc.tensor.matmul(
                            yB[:, lo:512],
                            v_sb[:, kt, 2 * pr + 1, :],
                            e[:, 1, lo:512],
                            start=(kt == 0),
                            stop=(kt == nkt - 1),
                        )
                    # normalize: y / rowsum (sums live in row 64 = partition 64)
                    rc = rcp.tile([D + 1, 2, 512], F32, tag="rc")
                    nc.vector.reciprocal(rc[D : D + 1, 0, :], yA[D : D + 1, :])
                    nc.vector.reciprocal(rc[D : D + 1, 1, :], yB[D : D + 1, :])
                    bc = bcp.tile([D, 2, 512], F32, tag="bc")
                    for h in (0, 1):
                        nc.sync.dma_start(
                            rcd[pr, qc, h : h + 1, :], rc[D : D + 1, h, :]
                        )
                        s = rcd[pr, qc, h, :]
                        src = bass.AP(
                            tensor=s.tensor,
                            offset=s.offset,
                            ap=[[0, D]] + list(s.ap),
                        )
                        nc.sync.dma_start(bc[:, h, :], src)
                    nc.vector.tensor_mul(
                        yT_all[0:D, pr, qs], yA[0:D, :], bc[:, 0, :]
                    )
                    # head B's rows must land on partitions 64..127; DVE is
                    # lane-aligned so stage at 0..63 and repartition via DMA
                    stg = stp.tile([D, 512], F32R, tag="stg")
                    nc.vector.tensor_mul(stg[:], yB[0:D, :], bc[:, 1, :])
                    nc.sync.dma_start(yT_all[64:128, pr, qs], stg[:])

            # output projection: out[t, c] = sum_j yT[j, t] * wpT[j, c]
            for ti in range(NT):
                tss = slice(ti * 128, ti * 128 + 128)
                for cc in range(2):
                    cs = slice(cc * 512, cc * 512 + 512)
                    acc = gp.tile([128, 2, 512], F32, tag="g")
                    for ji in range(NPAIR):
                        nc.tensor.matmul(
                            acc[:, 0, :],
                            yT_all[:, ji, tss],
                            wp_sb[:, ji, cs],
                            start=(ji == 0),
                            stop=(ji == NPAIR - 1),
                        )
                    o = outp.tile([128, 512], F32, tag="o")
                    nc.vector.tensor_copy(o[:], acc[:, 0, :])
                    nc.sync.dma_start(out[tss, cs], o[:])

    nc.finalize()
    return nc


def _get_nc():
    global _CACHED_NC
    if _CACHED_NC is None:
        _CACHED_NC = build_nc()
    return _CACHED_NC


def kernel(x, Wq, Wk, Wv, Wp):
    from concourse.bass_utils import run_bass_kernel_spmd

    x = np.asarray(x, dtype=np.float32)
    Wq = np.asarray(Wq, dtype=np.float32)
    Wk = np.asarray(Wk, dtype=np.float32)
    Wv = np.asarray(Wv, dtype=np.float32)
    Wp = np.asarray(Wp, dtype=np.float32)

    nc = _get_nc()

    xT = [np.ascontiguousarray(x[b].T) for b in range(B)]
    wqT, wkT, wvT, wpT = [], [], [], []
    for hh in range(2):
        js = slice(JL * hh, JL * hh + JL)
        wqT.append(np.ascontiguousarray(Wq[js, :].T))
        wkT.append(np.ascontiguousarray(Wk[js, :].T))
        wvT.append(np.ascontiguousarray(Wv[js, :].T))
        wpT.append(np.ascontiguousarray(Wp[:, js].T))

    in_maps = []
    for c in range(NCORES):
        b, hh = c // 2, c % 2
        in_maps.append(
            {
                "xT": xT[b],
                "wqT": wqT[hh],
                "wkT": wkT[hh],
                "wvT": wvT[hh],
                "wpT": wpT[hh],
            }
        )

    res = run_bass_kernel_spmd(nc, in_maps, core_ids=list(range(NCORES)))

    out = np.empty((B, T, C), dtype=np.float32)
    for b in range(B):
        out[b] = res.results[2 * b]["out"] + res.results[2 * b + 1]["out"]
    return out


# revision 7
# speedup vs baseline: 1.0822x; 1.0822x over previous
"""Causal self-attention Bass kernel for 8 TRN2 NeuronCores.

Problem: B=4, T=2048, C=1024, H=16 heads, head_dim=64, fp32.
    q = x @ Wq.T ; k = x @ Wk.T ; v = x @ Wv.T          (per head)
    att = softmax(mask(q k^T / 8))
    y = att @ v ; out = y @ Wp.T

Sharding (8 cores): 4-way data parallel over batch x 2-way tensor
parallel over heads. Core c handles batch c//2 and heads 8*(c%2)..+8.
Wq/Wk/Wv column-parallel, Wp row-parallel; the partial outputs of the
two head-halves of each batch are summed on the host (the "all-reduce"
of row-parallel Wp).

Device dataflow (all transposed, so no on-chip transposes are needed):
    xT [C, T] (host-pretransposed) ->
    qT/kT = WqT.T-slices @ xT   [512, T]  (pairs of heads on partitions)
    v     = xT.T-tiles @ WvT    [T, 512]
    scoresT[k, q] = kT.T @ qT   (k on partitions -> softmax sum over k
                                 via a ones-column appended to v)
    expT = exp(0.125 * scoresT) (no max subtraction: scores ~ N(0, 0.4))
    yT[d, q] (+ row of sums) = v_aug.T @ expT, accumulated over k tiles
    out[t, c] = yT.T-tiles @ WpT, accumulated over local j

Everything computes in float32r (TensorE full rate at N>=256,
~1.5e-4 rel err vs fp32).
"""

from contextlib import ExitStack

import numpy as np

import concourse.bass as bass
import concourse.tile as tile
from concourse import bacc, mybir

F32 = mybir.dt.float32
F32R = mybir.dt.float32r

B, T, C, H, D = 4, 2048, 1024, 16, 64
NCORES = 8
JL = 512            # local j dims per core (8 heads * 64)
NPAIR = 4           # local head pairs
CI = C // 128       # 8 c-tiles
NT = T // 128       # 16 t/k tiles
NQC = T // 512      # 4 q chunks
NEG = -1.0e30

_CACHED_NC = None


def build_nc():
    nc = bacc.Bacc(None)

    xT = nc.dram_tensor("xT", [C, T], F32R, kind="ExternalInput")
    wqT = nc.dram_tensor("wqT", [C, JL], F32R, kind="ExternalInput")
    wkT = nc.dram_tensor("wkT", [C, JL], F32R, kind="ExternalInput")
    wvT = nc.dram_tensor("wvT", [C, JL], F32R, kind="ExternalInput")
    wpT = nc.dram_tensor("wpT", [JL, C], F32R, kind="ExternalInput")
    out = nc.dram_tensor("out", [T, C], F32, kind="ExternalOutput")
    # bounce buffer for broadcasting softmax reciprocals across partitions
    rcd = nc.dram_tensor("rcd", [NPAIR, NQC, 2, 512], F32)

    xT_r = xT.rearrange("(ci p) t -> p ci t", p=128)
    wq_r = wqT.rearrange("(ci p) j -> p ci j", p=128)
    wk_r = wkT.rearrange("(ci p) j -> p ci j", p=128)
    wv_r = wvT.rearrange("(ci p) j -> p ci j", p=128)
    wp_r = wpT.rearrange("(ji p) c -> p ji c", p=128)

    with tile.TileContext(nc) as tc, ExitStack() as ctx:
        pm = ctx.enter_context(tc.tile_pool(name="pm", bufs=1))
        qkp = ctx.enter_context(tc.tile_pool(name="qkp", bufs=1))
        expp = ctx.enter_context(tc.tile_pool(name="expp", bufs=3))
        bcp = ctx.enter_context(tc.tile_pool(name="bcp", bufs=1))
        rcp = ctx.enter_context(tc.tile_pool(name="rcp", bufs=1))
        stp = ctx.enter_context(tc.tile_pool(name="stp", bufs=2))
        gp = ctx.enter_context(tc.tile_pool(name="gp", bufs=2, space="PSUM"))
        yp = ctx.enter_context(tc.tile_pool(name="yp", bufs=4, space="PSUM"))

        # v with a ones column appended per head: [k-tile, head, 65]
        v_sb = pm.tile([128, NT, 8, D + 1], F32R)
        # causal triangle mask for diagonal 128x128 blocks of scoresT:
        # mask[p, q] = 0 where q >= p else -1e30  (k on partitions)
        mask = pm.tile([128, 128], F32)
        nc.gpsimd.memset(mask[:], 0.0)
        nc.gpsimd.affine_select(
            out=mask[:],
            in_=mask[:],
            compare_op=mybir.AluOpType.is_ge,
            fill=NEG,
            base=0,
            pattern=[[1, 128]],
            channel_multiplier=-1,
        )
        ones_col = pm.tile([128, NT, 8, 1], F32)
        nc.vector.memset(ones_col[:], 1.0)
        nc.vector.tensor_copy(v_sb[:, :, :, D : D + 1], ones_col[:])

        qT_all = qkp.tile([128, NPAIR, T], F32R, tag="qT_all")
        kT_all = qkp.tile([128, NPAIR, T], F32R, tag="kT_all")

        # ---- phase 1: projections (stream xT by t-chunks) -----------------
        with (
            tc.tile_pool(name="ph1w", bufs=1) as wpool,
            tc.tile_pool(name="ph1x", bufs=2) as xpool,
        ):
            wq_sb = wpool.tile([128, CI, JL], F32R, tag="wq")
            wk_sb = wpool.tile([128, CI, JL], F32R, tag="wk")
            wv_sb = wpool.tile([128, CI, JL], F32R, tag="wv")
            nc.sync.dma_start(wq_sb[:], wq_r[:])
            nc.sync.dma_start(wk_sb[:], wk_r[:])
            nc.sync.dma_start(wv_sb[:], wv_r[:])

            for tch in range(NQC):
                ts_ = slice(tch * 512, tch * 512 + 512)
                xt = xpool.tile([128, CI, 512], F32R, tag="xt")
                nc.sync.dma_start(xt[:], xT_r[:, :, ts_])

                for w_sb, dst in ((wq_sb, qT_all), (wk_sb, kT_all)):
                    for pr in range(NPAIR):
                        acc = gp.tile([128, 2, 512], F32, tag="g")
                        for ci in range(CI):
                            nc.tensor.matmul(
                                acc[:, 0, :],
                                w_sb[:, ci, pr * 128 : pr * 128 + 128],
                                xt[:, ci, :],
                                start=(ci == 0),
                                stop=(ci == CI - 1),
                            )
                        nc.vector.tensor_copy(dst[:, pr, ts_], acc[:, 0, :])

                for tl in range(4):
                    ti = tch * 4 + tl
                    acc = gp.tile([128, 2, 512], F32, tag="g")
                    for ci in range(CI):
                        nc.tensor.matmul(
                            acc[:, 0, :],
                            xt[:, ci, tl * 128 : tl * 128 + 128],
                            wv_sb[:, ci, :],
                            start=(ci == 0),
                            stop=(ci == CI - 1),
                        )
                    nc.vector.tensor_copy(
                        v_sb[:, ti, :, 0:D],
                        acc[:, 0, :].rearrange("p (h d) -> p h d", d=D),
                    )

        # ---- phase 2: attention + output projection ----------------------
        with (
            tc.tile_pool(name="ph2", bufs=1) as p2,
            tc.tile_pool(name="outp", bufs=3) as outp,
        ):
            wp_sb = p2.tile([128, NPAIR, C], F32R, tag="wp")
            nc.sync.dma_start(wp_sb[:], wp_r[:])
            yT_all = p2.tile([128, NPAIR, T], F32R, tag="yT")

            for pr in range(NPAIR):
                qlo = qT_all[0:64, pr, :]
                qhi = qT_all[64:128, pr, :]
                klo = kT_all[0:64, pr, :]
                khi = kT_all[64:128, pr, :]
                for qc in range(NQC):
                    nkt = 4 * qc + 4
                    qs = slice(qc * 512, qc * 512 + 512)
                    yA = yp.tile([D + 1, 512], F32, tag="y")
                    yB = yp.tile([D + 1, 512], F32, tag="y")

                    # software pipeline: issue scores/exp for kt before the PV
                    # matmuls of kt-1, so the PE never waits on ACT's exp
                    def emit_pv(kt, e, nkt=nkt):
                        dt = kt - 4 * qc
                        lo = dt * 128 if dt > 0 else 0
                        nc.tensor.matmul(
                            yA[:, lo:512],
                            v_sb[:, kt, 2 * pr, :],
                            e[:, 0, lo:512],
                            start=(kt == 0),
                            stop=(kt == nkt - 1),
                        )
                        nc.tensor.matmul(
                            yB[:, lo:512],
                            v_sb[:, kt, 2 * pr + 1, :],
                            e[:, 1, lo:512],
                            start=(kt == 0),
                            stop=(kt == nkt - 1),
                        )

                    prev = None
                    for kt in range(nkt):
                        dt = kt - 4 * qc
                        ks = slice(kt * 128, kt * 128 + 128)
                        g = gp.tile([128, 2, 512], F32, tag="g")
                        nc.tensor.matmul(
                            g[:, 0, :], klo[:, ks], qlo[:, qs], start=True, stop=True
                        )
                        nc.tensor.matmul(
                            g[:, 1, :], khi[:, ks], qhi[:, qs], start=True, stop=True
                        )
                        if dt >= 0:
                            bs = slice(dt * 128, dt * 128 + 128)
                            nc.vector.tensor_add(g[:, 0, bs], g[:, 0, bs], mask[:])
                            nc.vector.tensor_add(g[:, 1, bs], g[:, 1, bs], mask[:])
                        e = expp.tile([128, 2, 512], F32R, tag="e")
                        nc.scalar.activation(
                            e[:], g[:], mybir.ActivationFunctionType.Exp, scale=0.125
                        )
                        if prev is not None:
                            emit_pv(*prev)
                        prev = (kt, e)
                    emit_pv(*prev)
                    # normalize: y / rowsum (sums live in row 64 = partition 64)
                    rc = rcp.tile([D + 1, 2, 512], F32, tag="rc")
                    nc.vector.reciprocal_approx_fast(rc[D : D + 1, 0, :], yA[D : D + 1, :])
                    nc.vector.reciprocal_approx_fast(rc[D : D + 1, 1, :], yB[D : D + 1, :])
                    bc = bcp.tile([D, 2, 512], F32, tag="bc")
                    for h in (0, 1):
                        nc.sync.dma_start(
                            rcd[pr, qc, h : h + 1, :], rc[D : D + 1, h, :]
                        )
                        s = rcd[pr, qc, h, :]
                        src = bass.AP(
                            tensor=s.tensor,
                            offset=s.offset,
                            ap=[[0, D]] + list(s.ap),
                        )
                        nc.sync.dma_start(bc[:, h, :], src)
                    nc.vector.tensor_mul(
                        yT_all[0:D, pr, qs], yA[0:D, :], bc[:, 0, :]
                    )
                    # head B's rows must land on partitions 64..127; DVE is
                    # lane-aligned so stage at 0..63 and repartition via DMA
                    stg = stp.tile([D, 512], F32R, tag="stg")
                    nc.vector.tensor_mul(stg[:], yB[0:D, :], bc[:, 1, :])
                    nc.sync.dma_start(yT_all[64:128, pr, qs], stg[:])

            # output projection: out[t, c] = sum_j yT[j, t] * wpT[j, c]
            for ti in range(NT):
                tss = slice(ti * 128, ti * 128 + 128)
                for cc in range(2):
                    cs = slice(cc * 512, cc * 512 + 512)
                    acc = gp.tile([128, 2, 512], F32, tag="g")
                    for ji in range(NPAIR):
                        nc.tensor.matmul(
                            acc[:, 0, :],
                            yT_all[:, ji, tss],
                            wp_sb[:, ji, cs],
                            start=(ji == 0),
                            stop=(ji == NPAIR - 1),
                        )
                    o = outp.tile([128, 512], F32, tag="o")
                    nc.vector.tensor_copy(o[:], acc[:, 0, :])
                    nc.sync.dma_start(out[tss, cs], o[:])

    nc.finalize()
    return nc


def _get_nc():
    global _CACHED_NC
    if _CACHED_NC is None:
        _CACHED_NC = build_nc()
    return _CACHED_NC


def kernel(x, Wq, Wk, Wv, Wp):
    from concourse.bass_utils import run_bass_kernel_spmd

    x = np.asarray(x, dtype=np.float32)
    Wq = np.asarray(Wq, dtype=np.float32)
    Wk = np.asarray(Wk, dtype=np.float32)
    Wv = np.asarray(Wv, dtype=np.float32)
    Wp = np.asarray(Wp, dtype=np.float32)

    nc = _get_nc()

    xT = [np.ascontiguousarray(x[b].T) for b in range(B)]
    wqT, wkT, wvT, wpT = [], [], [], []
    for hh in range(2):
        js = slice(JL * hh, JL * hh + JL)
        wqT.append(np.ascontiguousarray(Wq[js, :].T))
        wkT.append(np.ascontiguousarray(Wk[js, :].T))
        wvT.append(np.ascontiguousarray(Wv[js, :].T))
        wpT.append(np.ascontiguousarray(Wp[:, js].T))

    in_maps = []
    for c in range(NCORES):
        b, hh = c // 2, c % 2
        in_maps.append(
            {
                "xT": xT[b],
                "wqT": wqT[hh],
                "wkT": wkT[hh],
                "wvT": wvT[hh],
                "wpT": wpT[hh],
            }
        )

    res = run_bass_kernel_spmd(nc, in_maps, core_ids=list(range(NCORES)))

    out = np.empty((B, T, C), dtype=np.float32)
    for b in range(B):
        out[b] = res.results[2 * b]["out"] + res.results[2 * b + 1]["out"]
    return out
